# revision 2
# baseline (speedup 1.0000x reference)
"""Trainium2 Bass kernel v3 for nn_CurvatureLoss.

Data-parallel over batch (8 samples -> 8 cores). Per core: 9 overlapping
128-row slabs; per slab: softmax -> 3 prob maps -> curvature chain ->
masked accumulation. Design rules (from TimelineSim cost model):
  - single activation table (exp/ln/copy/square only -> 1 load total)
  - no scalar_tensor_tensor / reciprocal on the hot path (fp32-rate)
  - PSUM evacuation via ACT copy (scale folds constants); Pool multiplies
    PSUM-resident hyy directly; DVE does fp16 2x tensor_tensor + 4x
    tensor_scalar ops
  - D^-1.5 = exp(-1.5*ln(D)) on ACT (same table as softmax exp)
  - 2x scale cascade: lap2=2*lap so num4 = 4*num accumulates exactly
"""
import sys

if "/opt/trn_rl_repo" not in sys.path:
    sys.path.insert(0, "/opt/trn_rl_repo")

import numpy as np

P = 128
H = W = 1024
N_CORES = 8
STARTS = [0, 122, 244, 366, 488, 610, 732, 854, 896]
NSLAB = len(STARTS)
ACC_COLS = NSLAB * 3 * 2
SQ2 = 1.4142135623730951
HSQ2 = 0.7071067811865476


def _band_weights():
    """fp16 lhsT weights [128, 4*128]: M1.T, M2.T, M3.T, I."""
    SyP = np.eye(P, k=1, dtype=np.float64)   # (S+ x)[h] = x[h+1]
    SyM = np.eye(P, k=-1, dtype=np.float64)
    I = np.eye(P, dtype=np.float64)
    M1 = SyP + SyM - 4 * I
    M2 = SyP - SyM
    M3 = (2 * I - SyP - SyM) @ M2
    wts = np.concatenate([M1.T, M2.T, M3.T, I], axis=1).astype(np.float16)
    return np.ascontiguousarray(wts)


def _row_masks():
    masks = np.zeros((P, 3), np.float32)
    masks[0:125, 0] = -1.0
    masks[3:125, 1] = -1.0
    masks[83:128, 2] = -1.0
    return masks


_CACHE = {}

# mega tile type indices (GX,GY adjacent for the paired square; GY,HYY
# adjacent so one ACT copy evacuates both from PSUM)
T_LAP, T_GX, T_GY, T_HYY, T_USQ, T_VSQ, T_SQX, T_SQY, T_S, T_G2, T_G, T_T, \
    T_T2, T_DX = range(14)
NT = 14
CW = 1026  # chunk width (1024 + 2 pad cols)


def _build_program():
    import concourse.bacc as bacc
    import concourse.mybir as mybir
    from concourse.tile import TileContext

    f32 = mybir.dt.float32
    f16 = mybir.dt.float16
    Alu = mybir.AluOpType
    Act = mybir.ActivationFunctionType

    nc = bacc.Bacc("TRN2", target_bir_lowering=False, debug=False,
                   enable_asserts=False, num_devices=N_CORES)
    pred = nc.dram_tensor("pred", [4, H, W], f32, kind="ExternalInput").ap()
    wts = nc.dram_tensor("wts", [P, 4 * P], f16, kind="ExternalInput").ap()
    msk = nc.dram_tensor("msk", [P, 3], f32, kind="ExternalInput").ap()
    accd = nc.dram_tensor("acc", [P, ACC_COLS], f32, kind="ExternalOutput").ap()

    with TileContext(nc) as tc:
        with tc.tile_pool(name="const", bufs=1) as cpool, \
             tc.tile_pool(name="inp", bufs=3) as ipool, \
             tc.tile_pool(name="ps1", bufs=2, space="PSUM") as pp1, \
             tc.tile_pool(name="ps2", bufs=1, space="PSUM") as pp2, \
             nc.allow_low_precision(reason="fp16 chain validated vs reference"):
            # pin the act table: set 6 (natural_log_exp_and_others) holds
            # exp/ln/copy/square/relu -> exactly one table load total
            nc.scalar.add_instruction(mybir.InstLoadActFuncSet(
                name=nc.get_next_instruction_name(), act_func_set_id=6,
                ins=[], outs=[]))
            wt = cpool.tile([P, 4 * P], f16)
            nc.sync.dma_start(out=wt[:], in_=wts)
            w1 = wt[:, 0:P]
            w2 = wt[:, P:2 * P]
            w3 = wt[:, 2 * P:3 * P]
            wI = wt[:, 3 * P:4 * P]
            mtile = cpool.tile([P, 3], f32)
            nc.sync.dma_start(out=mtile[:], in_=msk)
            acc = cpool.tile([P, ACC_COLS], f32)
            nc.vector.memset(acc[:], 0.0)
            b_sq2 = cpool.tile([P, 1], f32)
            nc.vector.memset(b_sq2[:], SQ2)
            b_one = cpool.tile([P, 1], f32)
            nc.vector.memset(b_one[:], 1.0)

            mega = cpool.tile([P, NT, 3, CW], f16)
            # zero pad columns once (interior writes keep them zero)
            nc.gpsimd.memset(mega[:, :, :, 0:1], 0.0)
            nc.gpsimd.memset(mega[:, :, :, CW - 1:CW], 0.0)

            for si, st in enumerate(STARTS):
                mk = mtile[:, (0 if si == 0 else (2 if si == NSLAB - 1 else 1))
                           ][:, None]
                # ---- softmax: class DMA order [1,0,2,3] so chunk pairs
                # (0,2)+(1,3) give [e1+e2, e0+e3] in one op
                xt = ipool.tile([P, 4, W], f32, tag="xt")
                for k, cls in enumerate((1, 0, 2, 3)):
                    nc.sync.dma_start(out=xt[:, k, :],
                                      in_=pred[cls, st:st + P, :])
                ex = ipool.tile([P, 4, W], f16, tag="ex")
                nc.scalar.activation(out=ex[:, :, :], in_=xt[:, :, :],
                                     func=Act.Exp)
                sm = ipool.tile([P, 2, W], f16, tag="sm")  # [t12, s03]
                nc.vector.tensor_tensor(out=sm[:, :, :], in0=ex[:, 0:2, :],
                                        in1=ex[:, 2:4, :], op=Alu.add)
                tt = ipool.tile([P, W], f16, tag="tt")
                nc.vector.tensor_tensor(out=tt, in0=sm[:, 0, :],
                                        in1=sm[:, 1, :], op=Alu.add)
                nc.vector.reciprocal(out=tt, in_=tt)
                probs = ipool.tile([P, 3, CW], f16, tag="probs")
                nc.gpsimd.memset(probs[:, :, 0:1], 0.0)
                nc.gpsimd.memset(probs[:, :, CW - 1:CW], 0.0)
                nc.vector.tensor_tensor(out=probs[:, 0, 1:W + 1],
                                        in0=ex[:, 0, :], in1=tt, op=Alu.mult)
                nc.vector.tensor_tensor(out=probs[:, 1, 1:W + 1],
                                        in0=sm[:, 0, :], in1=tt, op=Alu.mult)
                nc.vector.tensor_tensor(out=probs[:, 2, 1:W + 1],
                                        in0=ex[:, 3, :], in1=tt, op=Alu.mult)

                # ---- per-map: PE stencils + evacuations
                for m in range(3):
                    pC = probs[:, m, 1:W + 1]
                    pE = probs[:, m, 2:W + 2]
                    pW_ = probs[:, m, 0:W]
                    ps_lap = pp1.tile([P, W], f32, tag="lap")
                    for hf in range(2):
                        sl = slice(hf * 512, (hf + 1) * 512)
                        nc.tensor.matmul(ps_lap[:, sl], lhsT=w1, rhs=pC[:, sl],
                                         start=True, stop=False)
                        nc.tensor.matmul(ps_lap[:, sl], lhsT=wI, rhs=pE[:, sl],
                                         start=False, stop=False)
                        nc.tensor.matmul(ps_lap[:, sl], lhsT=wI, rhs=pW_[:, sl],
                                         start=False, stop=True)
                    # lap2 = 2*lap
                    nc.scalar.activation(out=mega[:, T_LAP, m, 1:W + 1],
                                         in_=ps_lap[:], func=Act.Copy,
                                         scale=2.0)
                    lC = mega[:, T_LAP, m, 1:W + 1]
                    lE = mega[:, T_LAP, m, 2:W + 2]
                    lW = mega[:, T_LAP, m, 0:W]
                    # gx' = 2*gx
                    nc.vector.tensor_tensor(out=mega[:, T_GX, m, 1:W + 1],
                                            in0=lE, in1=lW, op=Alu.subtract)
                    ps_gh = pp2.tile([P, 2, W], f32, tag="gh")
                    for hf in range(2):
                        sl = slice(hf * 512, (hf + 1) * 512)
                        nc.tensor.matmul(ps_gh[:, 0, sl], lhsT=w2,
                                         rhs=lC[:, sl], start=True, stop=True)
                        nc.tensor.matmul(ps_gh[:, 1, sl], lhsT=w3,
                                         rhs=lC[:, sl], start=True, stop=True)
                    # [gy2, hyy2] = 2*[gy, hyy] in one PSUM->SBUF copy
                    nc.scalar.activation(
                        out=mega[:, T_GY:T_HYY + 1, m, 1:W + 1],
                        in_=ps_gh[:, :, :], func=Act.Copy)
                    # usq' = 2(1+gx)^2, vsq' = 2(1+gy)^2 in one paired op
                    nc.scalar.activation(
                        out=mega[:, T_USQ:T_VSQ + 1, m, 1:W + 1],
                        in_=mega[:, T_GX:T_GY + 1, m, 1:W + 1],
                        func=Act.Square, scale=HSQ2, bias=b_sq2[:])
                    # t2 = hyy2 * usq' = 4*hyy*(1+gx)^2  (GPSIMD is SBUF-only)
                    nc.gpsimd.tensor_tensor(out=mega[:, T_T2, m, 1:W + 1],
                                            in0=mega[:, T_HYY, m, 1:W + 1],
                                            in1=mega[:, T_USQ, m, 1:W + 1],
                                            op=Alu.mult)

                # ---- batched across 3 maps (views [3,1024])
                def V(t, lo=1, hi=W + 1):
                    return mega[:, t, :, lo:hi]

                gxC, gxE, gxW = V(T_GX), V(T_GX, 2, W + 2), V(T_GX, 0, W)
                nc.vector.tensor_tensor(out=V(T_S), in0=gxE, in1=gxW,
                                        op=Alu.add)
                nc.vector.tensor_scalar(out=V(T_G2), in0=gxC, scalar1=2.0,
                                        scalar2=None, op0=Alu.mult)
                # hxx2 = 2*hxx  (g2 - S)
                nc.vector.tensor_tensor(out=V(T_S), in0=V(T_G2), in1=V(T_S),
                                        op=Alu.subtract)
                # Dxv = 2*hxy
                nc.vector.tensor_tensor(out=V(T_DX), in0=gxE, in1=gxW,
                                        op=Alu.subtract)
                # G = gx'*gy2 = 4 gx gy ; Gd = G*Dxv = 8 hxy gx gy
                nc.vector.tensor_tensor(out=V(T_G), in0=gxC, in1=V(T_GY),
                                        op=Alu.mult)
                nc.vector.tensor_tensor(out=V(T_G), in0=V(T_G), in1=V(T_DX),
                                        op=Alu.mult)
                # t1 = hxx2*vsq' = 4 hxx (1+gy)^2
                nc.vector.tensor_tensor(out=V(T_T), in0=V(T_S), in1=V(T_VSQ),
                                        op=Alu.mult)
                # num4 = t1 + t2 - Gd
                nc.gpsimd.tensor_tensor(out=V(T_T), in0=V(T_T), in1=V(T_T2),
                                        op=Alu.add)
                nc.gpsimd.tensor_tensor(out=V(T_T), in0=V(T_T), in1=V(T_G),
                                        op=Alu.subtract)
                # D2 = 4(gx^2+gy^2)
                nc.vector.tensor_tensor(out=V(T_SQX), in0=gxC, in1=gxC,
                                        op=Alu.mult)
                nc.vector.tensor_tensor(out=V(T_SQY), in0=V(T_GY),
                                        in1=V(T_GY), op=Alu.mult)
                nc.vector.tensor_tensor(out=V(T_SQX), in0=V(T_SQX),
                                        in1=V(T_SQY), op=Alu.add)
                # rs3 = D^-1.5 via ln/exp (same act table as softmax exp)
                nc.scalar.activation(out=V(T_SQX), in_=V(T_SQX), func=Act.Ln,
                                     scale=0.25, bias=b_one[:])
                nc.scalar.activation(out=V(T_SQX), in_=V(T_SQX), func=Act.Exp,
                                     scale=-1.5)
                # z = num4 * rs3 = 8*curv
                nc.vector.tensor_tensor(out=V(T_T), in0=V(T_T), in1=V(T_SQX),
                                        op=Alu.mult)
                # masked relu + accum (s), then count (c) per map
                for m in range(3):
                    col = (si * 3 + m) * 2
                    zv = mega[:, T_T, m, 1:W + 1]
                    if m == 0:
                        # DVE path: mask*z, relu via max; separate sum-accum
                        nc.vector.tensor_scalar(
                            out=zv, in0=zv, scalar1=mk, scalar2=0.0,
                            op0=Alu.mult, op1=Alu.max)
                        nc.vector.tensor_scalar(
                            out=zv, in0=zv, scalar1=1.0, scalar2=None,
                            op0=Alu.mult, op1=Alu.add,
                            accum_out=acc[:, col:col + 1])
                    else:
                        nc.scalar.activation(
                            out=zv, in_=zv, func=Act.Relu, scale=mk,
                            accum_out=acc[:, col:col + 1])
                    nc.vector.tensor_scalar(
                        out=zv, in0=zv, scalar1=0.0, scalar2=None,
                        op0=Alu.is_gt, op1=Alu.add,
                        accum_out=acc[:, col + 1:col + 2])

            nc.sync.dma_start(out=accd, in_=acc[:])
    nc.compile()
    return nc


def _get_program():
    if "nc" not in _CACHE:
        _CACHE["nc"] = _build_program()
    return _CACHE["nc"]


def _run_device(pred_np):
    from concourse import bass_utils
    nc = _get_program()
    wts = _band_weights()
    msk = _row_masks()
    in_maps = [{"pred": np.ascontiguousarray(pred_np[b]), "wts": wts,
                "msk": msk}
               for b in range(N_CORES)]
    res = bass_utils.run_bass_kernel_spmd(nc, in_maps,
                                          core_ids=list(range(N_CORES)))
    return [res.results[b]["acc"] for b in range(N_CORES)]


def _host_reduce(accs):
    total = 0.0
    for b in range(N_CORES):
        a = accs[b].astype(np.float64)
        for m in range(3):
            s = a[:, [(si * 3 + m) * 2 for si in range(NSLAB)]].sum() / 8.0
            c = a[:, [(si * 3 + m) * 2 + 1 for si in range(NSLAB)]].sum()
            if c > 0:
                total += s / max(c, 1.0)
    return np.float32(total)


def kernel(pred, target=None):
    assert pred.shape == (N_CORES, 4, H, W)
    accs = _run_device(np.asarray(pred, dtype=np.float32))
    return _host_reduce(accs)
